# revision 12
# baseline (speedup 1.0000x reference)
"""GCNRouting2Hop on 8 trn2 NeuronCores (Bass/Tile SPMD kernel).

Sharding: dst-node partition (2500 nodes/core). Per core, edges (incl.
self-loops) are sorted by dst; per 128-dst block the chunk count is
padded to the max across cores so one SPMD program fits all cores, but
gathers skip pad rows at runtime (idx tails are -1 and the row count is
a per-core register loaded from SBUF).

Layer 1 aggregates x (128-dim) before the linear transform
(A@(x@W1) == (A@x)@W1). Gathered source rows (bf16) arrive via
dma_gather as [128 edges, D] tiles; host-precomputed one-hot scatter
tiles (iota==dstrel)*norm in bf16 stream in sequentially, and the
TensorEngine accumulates zT += gx.T @ onehot in PSUM. Dense matmuls
(weights bf16) + biases (folded into PSUM via K=1 ones-row matmuls) +
LayerNorm (ACT accumulators + Identity scale/bias) run per block.
h is cast to bf16 and AllGathered; layer 2 gathers h rows in bf16 and
repeats with K-split matmuls (256 features).
"""
import os
import sys
import types

sys.path.insert(0, '/opt/trn_rl_repo')
import numpy as np


def _install_axon_hooks_shim():
    try:
        import antenv
    except ImportError:
        return
    if hasattr(antenv, 'axon_hooks') or 'antenv.axon_hooks' in sys.modules:
        return
    try:
        from trn_agent_boot.trn_boot import _ntff_profile_via_ctypes
        hook = _ntff_profile_via_ctypes('/opt/axon/libaxon_pjrt.so')
    except Exception:
        hook = None
    mod = types.ModuleType('antenv.axon_hooks')
    mod._hook = hook
    mod.get_axon_ntff_profile_hook = lambda: mod._hook

    def set_axon_ntff_profile_hook(h):
        mod._hook = h

    mod.set_axon_ntff_profile_hook = set_axon_ntff_profile_hook
    sys.modules['antenv.axon_hooks'] = mod
    antenv.axon_hooks = mod


_install_axon_hooks_shim()

import ml_dtypes
from concourse import bacc, mybir, tile
from concourse.masks import make_identity
from concourse.bass_utils import run_bass_kernel_spmd

N = 20000
NC = 8
NPC = N // NC              # 2500 dst nodes per core
NBLK = (NPC + 127) // 128  # 20 blocks of 128 dst nodes
DIN = 128
DH = 256
LN_EPS = 1e-5

LAST_EXEC_TIME_NS = None
_prog_cache = {}

f32 = mybir.dt.float32
bf16 = mybir.dt.bfloat16
i16 = mybir.dt.int16
i32 = mybir.dt.int32

# cstf fp32 [128, 8, 256] rows; cstb bf16 [128, 3, 256]
(F_WRES, F_B1, F_BRES, F_G1, F_BE1, F_B2, F_G2, F_BE2) = range(8)
(B_W1, B_W2A, B_W2B, B_B1, B_BRES, B_B2) = range(6)


def _ln(nc, epi, u, gt, bt, out_tile, eps_ap):
    """LayerNorm over free axis; nodes on partitions. DVE kept off the
    slow TensorScalarPtr path: reductions on ACT accumulators, the
    normalize on ACT Identity with per-partition scale/bias."""
    sq = epi.tile([128, DH], f32, tag="sq")
    s1 = epi.tile([128, 1], f32, tag="s1")
    s2 = epi.tile([128, 1], f32, tag="s2")
    nc.scalar.activation(sq[:], u[:], mybir.ActivationFunctionType.Copy,
                         accum_out=s1[:])
    nc.scalar.activation(sq[:], u[:], mybir.ActivationFunctionType.Square,
                         accum_out=s2[:])
    mu = epi.tile([128, 1], f32, tag="mu")
    nc.vector.tensor_scalar(out=mu[:], in0=s1[:], scalar1=1.0 / DH,
                            scalar2=None, op0=mybir.AluOpType.mult)
    var = epi.tile([128, 1], f32, tag="var")
    musq = epi.tile([128, 1], f32, tag="musq")
    nc.vector.tensor_tensor(out=musq[:], in0=mu[:], in1=mu[:],
                            op=mybir.AluOpType.mult)
    nc.vector.tensor_scalar(out=var[:], in0=s2[:], scalar1=1.0 / DH,
                            scalar2=None, op0=mybir.AluOpType.mult)
    nc.vector.tensor_tensor(out=var[:], in0=var[:], in1=musq[:],
                            op=mybir.AluOpType.subtract)
    std = epi.tile([128, 1], f32, tag="std")
    nc.scalar.activation(std[:], var[:], mybir.ActivationFunctionType.Sqrt,
                         bias=eps_ap)
    rstd = epi.tile([128, 1], f32, tag="rstd")
    nc.vector.reciprocal(rstd[:], std[:])
    nmr = epi.tile([128, 1], f32, tag="nmr")
    nc.vector.tensor_tensor(out=nmr[:], in0=mu[:], in1=rstd[:],
                            op=mybir.AluOpType.mult)
    nc.vector.tensor_scalar(out=nmr[:], in0=nmr[:], scalar1=-1.0,
                            scalar2=None, op0=mybir.AluOpType.mult)
    un = epi.tile([128, DH], f32, tag="un")
    nc.scalar.activation(un[:], u[:], mybir.ActivationFunctionType.Identity,
                         bias=nmr[:], scale=rstd[:])
    g = epi.tile([128, DH], f32, tag="g")
    nc.vector.tensor_tensor(out=g[:], in0=un[:], in1=gt,
                            op=mybir.AluOpType.mult)
    nc.vector.tensor_tensor(out=out_tile[:], in0=g[:], in1=bt,
                            op=mybir.AluOpType.add)


def _build_program(R, stage="full"):
    offs = np.concatenate([[0], np.cumsum(R)]).astype(np.int64)
    E_pad = int(offs[-1])
    CH = E_pad // 128
    smax = int(max(R)) // 128

    nc = bacc.Bacc("TRN2", target_bir_lowering=False, debug=False,
                   num_devices=NC, num_swdge_queues=4)
    xbf_in = nc.dram_tensor("xbf", [N, DIN], bf16, kind="ExternalInput")
    idx_in = nc.dram_tensor("idx1", [128, E_pad // 16], i16,
                            kind="ExternalInput")
    oh_in = nc.dram_tensor("ohb", [128, CH, 128], bf16, kind="ExternalInput")
    xT_in = nc.dram_tensor("xT", [128, NBLK * 128], f32, kind="ExternalInput")
    cstf_in = nc.dram_tensor("cstf", [128, 8, DH], f32, kind="ExternalInput")
    cstb_in = nc.dram_tensor("cstb", [128, 6, DH], bf16, kind="ExternalInput")
    cnt_in = nc.dram_tensor("cnt", [1, NBLK], i32, kind="ExternalInput")
    out_t = nc.dram_tensor("out", [NPC, DH], f32, kind="ExternalOutput")

    with tile.TileContext(nc) as tc:
        with tc.tile_pool(name="keep", bufs=1) as keep, \
             tc.tile_pool(name="gxp", bufs=3) as gxp, \
             tc.tile_pool(name="ohp", bufs=2) as ohp, \
             tc.tile_pool(name="rot", bufs=2) as rot, \
             tc.tile_pool(name="epi", bufs=3) as epi, \
             tc.tile_pool(name="ps_zt", bufs=2, space="PSUM") as ps_zt, \
             tc.tile_pool(name="ps_dn", bufs=2, space="PSUM") as ps_dn, \
             tc.tile_pool(name="ps_ag", bufs=4, space="PSUM") as ps_ag, \
             tc.tile_pool(name="dram", bufs=1, space="DRAM") as dram:

            # ---- preload ----
            cstf = keep.tile([128, 8, DH], f32)
            nc.sync.dma_start(cstf[:], cstf_in[:])
            cstb = keep.tile([128, 6, DH], bf16)
            nc.sync.dma_start(cstb[:], cstb_in[:])
            idx1 = keep.tile([128, E_pad // 16], i16)
            nc.sync.dma_start(idx1[:], idx_in[:])
            xT = keep.tile([128, NBLK * 128], f32)
            nc.sync.dma_start(xT[:], xT_in[:])
            cnt_t = keep.tile([1, NBLK], i32)
            nc.sync.dma_start(cnt_t[:], cnt_in[:])
            _, cnt_vals = nc.values_load_multi_w_load_instructions(
                cnt_t[:], engines=(mybir.EngineType.Pool,),
                min_val=0, max_val=int(max(R)),
                skip_runtime_bounds_check=True)

            eps_t = keep.tile([128, 1], f32)
            nc.vector.memset(eps_t[:], LN_EPS)
            ones_t = keep.tile([1, 128], bf16)
            nc.vector.memset(ones_t[:], 1.0)
            ident = keep.tile([128, 128], bf16)
            make_identity(nc, ident[:])
            h_own = keep.tile([128, NBLK * DH], bf16)

            hg_self = dram.tile([NPC, DH], bf16)
            hg_full = dram.tile([N, DH], bf16)

            Wresf = cstf[:, F_WRES, :]
            g1t = cstf[:, F_G1, :]
            be1t = cstf[:, F_BE1, :]
            g2t = cstf[:, F_G2, :]
            be2t = cstf[:, F_BE2, :]
            b1row = cstb[0:1, B_B1, :]
            bresrow = cstb[0:1, B_BRES, :]
            b2row = cstb[0:1, B_B2, :]
            W1b = cstb[:, B_W1, :]
            W2ab = cstb[:, B_W2A, :]
            W2bb = cstb[:, B_W2B, :]

            # ---- layer 1 ----
            for b in range(NBLK):
                nchunk = R[b] // 128
                o16 = int(offs[b]) // 16
                t0 = int(offs[b]) // 128
                oht = ohp.tile([128, smax, 128], bf16, tag="oh1")
                nc.sync.dma_start(oht[:, 0:nchunk, :],
                                  oh_in[:, t0:t0 + nchunk, :])
                gx = gxp.tile([128, smax, DIN], bf16, tag="gx")
                nc.gpsimd.dma_gather(
                    out_ap=gx[:, 0:nchunk, :], in_ap=xbf_in[:],
                    idxs_ap=idx1[:, o16:o16 + R[b] // 16],
                    num_idxs=R[b], num_idxs_reg=cnt_vals[b], elem_size=DIN,
                    single_packet=False, queue_num=b % 4)
                psum_zT = ps_zt.tile([128, 128], f32, tag="zt", space="PSUM")
                for k in range(nchunk):
                    nc.tensor.matmul(out=psum_zT[:], lhsT=gx[:, k, :],
                                     rhs=oht[:, k, :], start=(k == 0),
                                     stop=(k == nchunk - 1))
                zts = rot.tile([128, 128], bf16, tag="zts")
                nc.scalar.activation(zts[:], psum_zT[:],
                                     mybir.ActivationFunctionType.Copy)
                psum_h1 = ps_dn.tile([128, DH], f32, tag="dense",
                                     space="PSUM")
                nc.tensor.matmul(out=psum_h1[:], lhsT=ones_t[:], rhs=b1row,
                                 start=True, stop=False)
                nc.tensor.matmul(out=psum_h1[:], lhsT=zts[:], rhs=W1b,
                                 start=False, stop=True)
                psum_r = ps_dn.tile([128, DH], f32, tag="dense", space="PSUM")
                nc.tensor.matmul(out=psum_r[:], lhsT=ones_t[:], rhs=bresrow,
                                 start=True, stop=False)
                nc.tensor.matmul(out=psum_r[:],
                                 lhsT=xT[:, b * 128:(b + 1) * 128],
                                 rhs=Wresf, start=False, stop=True)
                delta = epi.tile([128, DH], f32, tag="delta")
                nc.scalar.activation(delta[:], psum_h1[:],
                                     mybir.ActivationFunctionType.Relu)
                u = epi.tile([128, DH], f32, tag="u")
                nc.vector.tensor_tensor(out=u[:], in0=psum_r[:],
                                        in1=delta[:], op=mybir.AluOpType.add)
                hblk = h_own[:, b * DH:(b + 1) * DH]
                _ln(nc, epi, u, g1t, be1t, hblk, eps_t[:])
                rows = min(128, NPC - b * 128)
                nc.sync.dma_start(
                    out=hg_self[b * 128:b * 128 + rows, :],
                    in_=h_own[0:rows, b * DH:(b + 1) * DH])

            if stage == "l1":
                for b in range(NBLK):
                    rows = min(128, NPC - b * 128)
                    nc.gpsimd.dma_start(
                        out=out_t[b * 128:b * 128 + rows, :],
                        in_=h_own[0:rows, b * DH:(b + 1) * DH])

            # ---- exchange ----
            if stage != "l1":
                nc.gpsimd.collective_compute(
                    "AllGather", mybir.AluOpType.bypass,
                    replica_groups=[list(range(NC))],
                    ins=[hg_self.opt()], outs=[hg_full.opt()])

            # ---- layer 2 ----
            for b in range(NBLK if stage == "full" else 0):
                nchunk = R[b] // 128
                o16 = int(offs[b]) // 16
                t0 = int(offs[b]) // 128
                oht = ohp.tile([128, smax, 128], bf16, tag="oh2")
                nc.sync.dma_start(oht[:, 0:nchunk, :],
                                  oh_in[:, t0:t0 + nchunk, :])
                gh = gxp.tile([128, smax, DH], bf16, tag="gh")
                nc.gpsimd.dma_gather(
                    out_ap=gh[:, 0:nchunk, :], in_ap=hg_full[:],
                    idxs_ap=idx1[:, o16:o16 + R[b] // 16],
                    num_idxs=R[b], num_idxs_reg=cnt_vals[b], elem_size=DH,
                    single_packet=False, queue_num=b % 4)
                psum_lo = ps_ag.tile([128, 128], f32, tag="agg", space="PSUM")
                psum_hi = ps_ag.tile([128, 128], f32, tag="agg", space="PSUM")
                for k in range(nchunk):
                    nc.tensor.matmul(out=psum_lo[:], lhsT=gh[:, k, 0:128],
                                     rhs=oht[:, k, :], start=(k == 0),
                                     stop=(k == nchunk - 1))
                    nc.tensor.matmul(out=psum_hi[:], lhsT=gh[:, k, 128:256],
                                     rhs=oht[:, k, :], start=(k == 0),
                                     stop=(k == nchunk - 1))
                z2lo = rot.tile([128, 128], bf16, tag="z2lo")
                nc.scalar.activation(z2lo[:], psum_lo[:],
                                     mybir.ActivationFunctionType.Copy)
                z2hi = rot.tile([128, 128], bf16, tag="z2hi")
                nc.scalar.activation(z2hi[:], psum_hi[:],
                                     mybir.ActivationFunctionType.Copy)
                psum_d2 = ps_dn.tile([128, DH], f32, tag="dense",
                                     space="PSUM")
                nc.tensor.matmul(out=psum_d2[:], lhsT=ones_t[:], rhs=b2row,
                                 start=True, stop=False)
                nc.tensor.matmul(out=psum_d2[:], lhsT=z2lo[:], rhs=W2ab,
                                 start=False, stop=False)
                nc.tensor.matmul(out=psum_d2[:], lhsT=z2hi[:], rhs=W2bb,
                                 start=False, stop=False)
                nc.tensor.matmul(out=psum_d2[:], lhsT=ident[:],
                                 rhs=h_own[:, b * DH:(b + 1) * DH],
                                 start=False, stop=True)
                outb = epi.tile([128, DH], f32, tag="outb")
                _ln(nc, epi, psum_d2, g2t, be2t, outb, eps_t[:])
                rows = min(128, NPC - b * 128)
                nc.sync.dma_start(out=out_t[b * 128:b * 128 + rows, :],
                                  in_=outb[0:rows, :])
    nc.compile()
    return nc


def _host_prep(edge_index, edge_weight):
    src = np.concatenate([np.asarray(edge_index[0], np.int64),
                          np.arange(N, dtype=np.int64)])
    dst = np.concatenate([np.asarray(edge_index[1], np.int64),
                          np.arange(N, dtype=np.int64)])
    w = np.concatenate([np.asarray(edge_weight, np.float32),
                        np.ones(N, np.float32)])
    deg = np.zeros(N, np.float32)
    np.add.at(deg, dst, w)
    dinv = np.where(deg > 0, 1.0 / np.sqrt(deg), 0.0).astype(np.float32)
    norm = (dinv[src] * w * dinv[dst]).astype(np.float32)

    order = np.argsort(dst, kind='stable')
    src_s, dst_s, norm_s = src[order], dst[order], norm[order]

    core_id = dst_s // NPC
    brel = (dst_s % NPC) // 128
    cnt = np.zeros((NC, NBLK), np.int64)
    np.add.at(cnt, (core_id, brel), 1)
    R = tuple(int(v) for v in (np.ceil(cnt.max(axis=0) / 128) * 128)
              .astype(np.int64))
    offs = np.concatenate([[0], np.cumsum(R)]).astype(np.int64)
    E_pad = int(offs[-1])

    src_pad = np.full((NC, E_pad), -1, np.int16)
    dstrel_pad = np.zeros((NC, E_pad), np.int64)
    wn_pad = np.zeros((NC, E_pad), np.float32)
    real = np.zeros((NC, E_pad), bool)
    for c in range(NC):
        for b in range(NBLK):
            lo = np.searchsorted(dst_s, c * NPC + b * 128, 'left')
            hi = np.searchsorted(
                dst_s, min(c * NPC + (b + 1) * 128, (c + 1) * NPC), 'left')
            n = hi - lo
            o = int(offs[b])
            src_pad[c, o:o + n] = src_s[lo:hi]
            dstrel_pad[c, o:o + n] = dst_s[lo:hi] - (c * NPC + b * 128)
            wn_pad[c, o:o + n] = norm_s[lo:hi]
            real[c, o:o + n] = True
    return R, cnt.astype(np.int32), src_pad, dstrel_pad, wn_pad, real


def kernel(x, edge_index, edge_weight, W1, b1, W2, b2, Wres, bres,
           gamma1, beta1, gamma2, beta2):
    global LAST_EXEC_TIME_NS
    x = np.ascontiguousarray(np.asarray(x, np.float32))
    W1 = np.asarray(W1, np.float32)
    W2 = np.asarray(W2, np.float32)
    Wres = np.asarray(Wres, np.float32)

    R, cnt, src_pad, dstrel_pad, wn_pad, real = _host_prep(
        edge_index, edge_weight)
    E_pad = int(sum(R))
    CH = E_pad // 128

    cstf = np.zeros((128, 8, DH), np.float32)
    cstf[:, F_WRES, :] = Wres
    cstf[:, F_B1, :] = np.asarray(b1, np.float32)[None, :]
    cstf[:, F_BRES, :] = np.asarray(bres, np.float32)[None, :]
    cstf[:, F_G1, :] = np.asarray(gamma1, np.float32)[None, :]
    cstf[:, F_BE1, :] = np.asarray(beta1, np.float32)[None, :]
    cstf[:, F_B2, :] = np.asarray(b2, np.float32)[None, :]
    cstf[:, F_G2, :] = np.asarray(gamma2, np.float32)[None, :]
    cstf[:, F_BE2, :] = np.asarray(beta2, np.float32)[None, :]
    cstb = np.zeros((128, 6, DH), np.float32)
    cstb[:, B_W1, :] = W1
    cstb[:, B_W2A, :] = W2[:128, :]
    cstb[:, B_W2B, :] = W2[128:, :]
    cstb[:, B_B1, :] = np.asarray(b1, np.float32)[None, :]
    cstb[:, B_BRES, :] = np.asarray(bres, np.float32)[None, :]
    cstb[:, B_B2, :] = np.asarray(b2, np.float32)[None, :]
    cstb = cstb.astype(ml_dtypes.bfloat16)

    xbf = x.astype(ml_dtypes.bfloat16)

    in_maps = []
    for c in range(NC):
        idx_w = np.tile(src_pad[c].reshape(E_pad // 16, 16).T, (8, 1)).copy()
        # one-hot scatter tiles: [128, CH, 128] bf16
        oh = np.zeros((E_pad, 128), np.float32)
        rr = real[c]
        oh[np.nonzero(rr)[0], dstrel_pad[c][rr]] = wn_pad[c][rr]
        oh = np.ascontiguousarray(
            oh.reshape(CH, 128, 128).transpose(1, 0, 2)
        ).astype(ml_dtypes.bfloat16)
        xT = np.zeros((128, NBLK * 128), np.float32)
        xT[:, :NPC] = x[c * NPC:(c + 1) * NPC].T
        in_maps.append({
            "xbf": xbf,
            "idx1": idx_w,
            "ohb": oh,
            "xT": xT,
            "cstf": cstf,
            "cstb": cstb,
            "cnt": cnt[c].reshape(1, NBLK),
        })

    stage = os.environ.get("GCN_STAGE", "full")
    key = (R, stage)
    nc = _prog_cache.get(key)
    if nc is None:
        nc = _build_program(R, stage)
        _prog_cache[key] = nc

    trace = bool(os.environ.get("BASS_KERNEL_TRACE"))
    res = run_bass_kernel_spmd(nc, in_maps, list(range(NC)), trace=trace)
    if trace:
        LAST_EXEC_TIME_NS = res.exec_time_ns
    out = np.concatenate([res.results[c]["out"] for c in range(NC)], axis=0)
    return np.ascontiguousarray(out.astype(np.float32))


# revision 13
# speedup vs baseline: 1.2436x; 1.2436x over previous
"""GCNRouting2Hop on 8 trn2 NeuronCores (Bass/Tile SPMD kernel).

Sharding: dst-node partition (2500 nodes/core). Per core, edges (incl.
self-loops) are sorted by dst; per 128-dst block the chunk count is
padded to the max across cores so one SPMD program fits all cores, but
gathers skip pad rows at runtime (idx tails are -1 and the row count is
a per-core register loaded from SBUF).

Layer 1 aggregates x (128-dim) before the linear transform
(A@(x@W1) == (A@x)@W1). Gathered source rows (bf16) arrive via
dma_gather as [128 edges, D] tiles; host-precomputed one-hot scatter
tiles (iota==dstrel)*norm in bf16 stream in sequentially, and the
TensorEngine accumulates zT += gx.T @ onehot in PSUM. Dense matmuls
(weights bf16) + biases (folded into PSUM via K=1 ones-row matmuls) +
LayerNorm (ACT accumulators + Identity scale/bias) run per block.
h is cast to bf16 and AllGathered; layer 2 gathers h rows in bf16 and
repeats with K-split matmuls (256 features).
"""
import os
import sys
import types

sys.path.insert(0, '/opt/trn_rl_repo')
import numpy as np


def _install_axon_hooks_shim():
    try:
        import antenv
    except ImportError:
        return
    if hasattr(antenv, 'axon_hooks') or 'antenv.axon_hooks' in sys.modules:
        return
    try:
        from trn_agent_boot.trn_boot import _ntff_profile_via_ctypes
        hook = _ntff_profile_via_ctypes('/opt/axon/libaxon_pjrt.so')
    except Exception:
        hook = None
    mod = types.ModuleType('antenv.axon_hooks')
    mod._hook = hook
    mod.get_axon_ntff_profile_hook = lambda: mod._hook

    def set_axon_ntff_profile_hook(h):
        mod._hook = h

    mod.set_axon_ntff_profile_hook = set_axon_ntff_profile_hook
    sys.modules['antenv.axon_hooks'] = mod
    antenv.axon_hooks = mod


_install_axon_hooks_shim()

import ml_dtypes
from concourse import bacc, mybir, tile
from concourse.masks import make_identity
from concourse.bass_utils import run_bass_kernel_spmd

N = 20000
NC = 8
NPC = N // NC              # 2500 dst nodes per core
NBLK = (NPC + 127) // 128  # 20 blocks of 128 dst nodes
DIN = 128
DH = 256
LN_EPS = 1e-5

LAST_EXEC_TIME_NS = None
_prog_cache = {}

f32 = mybir.dt.float32
bf16 = mybir.dt.bfloat16
i16 = mybir.dt.int16
i32 = mybir.dt.int32

# cstf fp32 [128, 8, 256] rows; cstb bf16 [128, 3, 256]
(F_WRES, F_B1, F_BRES, F_G1, F_BE1, F_B2, F_G2, F_BE2) = range(8)
(B_W1, B_W2A, B_W2B, B_B1, B_BRES, B_B2) = range(6)


def _ln(nc, epi, u, gt, bt, out_tile, eps_ap):
    """LayerNorm over free axis; nodes on partitions. DVE kept off the
    slow TensorScalarPtr path: reductions on ACT accumulators, the
    normalize on ACT Identity with per-partition scale/bias."""
    sq = epi.tile([128, DH], f32, tag="sq")
    s1 = epi.tile([128, 1], f32, tag="s1")
    s2 = epi.tile([128, 1], f32, tag="s2")
    nc.scalar.activation(sq[:], u[:], mybir.ActivationFunctionType.Copy,
                         accum_out=s1[:])
    nc.scalar.activation(sq[:], u[:], mybir.ActivationFunctionType.Square,
                         accum_out=s2[:])
    mu = epi.tile([128, 1], f32, tag="mu")
    nc.vector.tensor_scalar(out=mu[:], in0=s1[:], scalar1=1.0 / DH,
                            scalar2=None, op0=mybir.AluOpType.mult)
    var = epi.tile([128, 1], f32, tag="var")
    musq = epi.tile([128, 1], f32, tag="musq")
    nc.vector.tensor_tensor(out=musq[:], in0=mu[:], in1=mu[:],
                            op=mybir.AluOpType.mult)
    nc.vector.tensor_scalar(out=var[:], in0=s2[:], scalar1=1.0 / DH,
                            scalar2=None, op0=mybir.AluOpType.mult)
    nc.vector.tensor_tensor(out=var[:], in0=var[:], in1=musq[:],
                            op=mybir.AluOpType.subtract)
    std = epi.tile([128, 1], f32, tag="std")
    nc.scalar.activation(std[:], var[:], mybir.ActivationFunctionType.Sqrt,
                         bias=eps_ap)
    rstd = epi.tile([128, 1], f32, tag="rstd")
    nc.vector.reciprocal(rstd[:], std[:])
    nmr = epi.tile([128, 1], f32, tag="nmr")
    nc.vector.tensor_tensor(out=nmr[:], in0=mu[:], in1=rstd[:],
                            op=mybir.AluOpType.mult)
    nc.vector.tensor_scalar(out=nmr[:], in0=nmr[:], scalar1=-1.0,
                            scalar2=None, op0=mybir.AluOpType.mult)
    un = epi.tile([128, DH], f32, tag="un")
    nc.scalar.activation(un[:], u[:], mybir.ActivationFunctionType.Identity,
                         bias=nmr[:], scale=rstd[:])
    g = epi.tile([128, DH], f32, tag="g")
    nc.vector.tensor_tensor(out=g[:], in0=un[:], in1=gt,
                            op=mybir.AluOpType.mult)
    nc.vector.tensor_tensor(out=out_tile[:], in0=g[:], in1=bt,
                            op=mybir.AluOpType.add)


def _build_program(R, stage="full"):
    offs = np.concatenate([[0], np.cumsum(R)]).astype(np.int64)
    E_pad = int(offs[-1])
    CH = E_pad // 128
    smax = int(max(R)) // 128

    nc = bacc.Bacc("TRN2", target_bir_lowering=False, debug=False,
                   num_devices=NC, num_swdge_queues=4)
    xbf_in = nc.dram_tensor("xbf", [N, DIN], bf16, kind="ExternalInput")
    idx_in = nc.dram_tensor("idx1", [128, E_pad // 16], i16,
                            kind="ExternalInput")
    idx2_in = nc.dram_tensor("idx2", [128, E_pad // 16], i16,
                             kind="ExternalInput")
    oh_in = nc.dram_tensor("ohb", [128, CH, 128], bf16, kind="ExternalInput")
    xT_in = nc.dram_tensor("xT", [128, NBLK * 128], f32, kind="ExternalInput")
    cstf_in = nc.dram_tensor("cstf", [128, 8, DH], f32, kind="ExternalInput")
    cstb_in = nc.dram_tensor("cstb", [128, 6, DH], bf16, kind="ExternalInput")
    cnt_in = nc.dram_tensor("cnt", [1, NBLK], i32, kind="ExternalInput")
    out_t = nc.dram_tensor("out", [NPC, DH], f32, kind="ExternalOutput")

    with tile.TileContext(nc) as tc:
        with tc.tile_pool(name="keep", bufs=1) as keep, \
             tc.tile_pool(name="gxp", bufs=4) as gxp, \
             tc.tile_pool(name="ohp", bufs=2) as ohp, \
             tc.tile_pool(name="rot", bufs=2) as rot, \
             tc.tile_pool(name="epi", bufs=3) as epi, \
             tc.tile_pool(name="ps_zt", bufs=2, space="PSUM") as ps_zt, \
             tc.tile_pool(name="ps_dn", bufs=2, space="PSUM") as ps_dn, \
             tc.tile_pool(name="ps_ag", bufs=4, space="PSUM") as ps_ag, \
             tc.tile_pool(name="dram", bufs=1, space="DRAM") as dram:

            # ---- preload ----
            cstf = keep.tile([128, 8, DH], f32)
            nc.sync.dma_start(cstf[:], cstf_in[:])
            cstb = keep.tile([128, 6, DH], bf16)
            nc.sync.dma_start(cstb[:], cstb_in[:])
            idx1 = keep.tile([128, E_pad // 16], i16)
            nc.sync.dma_start(idx1[:], idx_in[:])
            idx2 = keep.tile([128, E_pad // 16], i16)
            nc.sync.dma_start(idx2[:], idx2_in[:])
            xT = keep.tile([128, NBLK * 128], f32)
            nc.sync.dma_start(xT[:], xT_in[:])
            cnt_t = keep.tile([1, NBLK], i32)
            nc.sync.dma_start(cnt_t[:], cnt_in[:])
            _, cnt_vals = nc.values_load_multi_w_load_instructions(
                cnt_t[:], engines=(mybir.EngineType.Pool,),
                min_val=0, max_val=int(max(R)),
                skip_runtime_bounds_check=True)

            eps_t = keep.tile([128, 1], f32)
            nc.vector.memset(eps_t[:], LN_EPS)
            ones_t = keep.tile([1, 128], bf16)
            nc.vector.memset(ones_t[:], 1.0)
            ident = keep.tile([128, 128], bf16)
            make_identity(nc, ident[:])
            h_own = keep.tile([128, NBLK * DH], bf16)

            hg_self = dram.tile([NPC, DH], bf16)
            hg_full = dram.tile([N, DH], bf16)
            # 4 exchange slices: blocks 0-4, 5-9, 10-14, 15-19
            SL = [640, 640, 640, NPC - 3 * 640]


            Wresf = cstf[:, F_WRES, :]
            g1t = cstf[:, F_G1, :]
            be1t = cstf[:, F_BE1, :]
            g2t = cstf[:, F_G2, :]
            be2t = cstf[:, F_BE2, :]
            b1row = cstb[0:1, B_B1, :]
            bresrow = cstb[0:1, B_BRES, :]
            b2row = cstb[0:1, B_B2, :]
            W1b = cstb[:, B_W1, :]
            W2ab = cstb[:, B_W2A, :]
            W2bb = cstb[:, B_W2B, :]

            # ---- layer 1 ----
            for b in range(NBLK):
                nchunk = R[b] // 128
                o16 = int(offs[b]) // 16
                t0 = int(offs[b]) // 128
                oht = ohp.tile([128, smax, 128], bf16, tag="oh1")
                nc.sync.dma_start(oht[:, 0:nchunk, :],
                                  oh_in[:, t0:t0 + nchunk, :])
                gx = gxp.tile([128, smax, DIN], bf16, tag="gx")
                nc.gpsimd.dma_gather(
                    out_ap=gx[:, 0:nchunk, :], in_ap=xbf_in[:],
                    idxs_ap=idx1[:, o16:o16 + R[b] // 16],
                    num_idxs=R[b], num_idxs_reg=cnt_vals[b], elem_size=DIN,
                    single_packet=False, queue_num=b % 4)
                psum_zT = ps_zt.tile([128, 128], f32, tag="zt", space="PSUM")
                for k in range(nchunk):
                    nc.tensor.matmul(out=psum_zT[:], lhsT=gx[:, k, :],
                                     rhs=oht[:, k, :], start=(k == 0),
                                     stop=(k == nchunk - 1))
                zts = rot.tile([128, 128], bf16, tag="zts")
                nc.scalar.activation(zts[:], psum_zT[:],
                                     mybir.ActivationFunctionType.Copy)
                psum_h1 = ps_dn.tile([128, DH], f32, tag="dense",
                                     space="PSUM")
                nc.tensor.matmul(out=psum_h1[:], lhsT=ones_t[:], rhs=b1row,
                                 start=True, stop=False)
                nc.tensor.matmul(out=psum_h1[:], lhsT=zts[:], rhs=W1b,
                                 start=False, stop=True)
                psum_r = ps_dn.tile([128, DH], f32, tag="dense", space="PSUM")
                nc.tensor.matmul(out=psum_r[:], lhsT=ones_t[:], rhs=bresrow,
                                 start=True, stop=False)
                nc.tensor.matmul(out=psum_r[:],
                                 lhsT=xT[:, b * 128:(b + 1) * 128],
                                 rhs=Wresf, start=False, stop=True)
                delta = epi.tile([128, DH], f32, tag="delta")
                nc.scalar.activation(delta[:], psum_h1[:],
                                     mybir.ActivationFunctionType.Relu)
                u = epi.tile([128, DH], f32, tag="u")
                nc.vector.tensor_tensor(out=u[:], in0=psum_r[:],
                                        in1=delta[:], op=mybir.AluOpType.add)
                hblk = h_own[:, b * DH:(b + 1) * DH]
                _ln(nc, epi, u, g1t, be1t, hblk, eps_t[:])
                rows = min(128, NPC - b * 128)
                nc.sync.dma_start(
                    out=hg_self[b * 128:b * 128 + rows, :],
                    in_=h_own[0:rows, b * DH:(b + 1) * DH])
                if stage != "l1" and (b + 1) % 5 == 0:
                    q = (b + 1) // 5 - 1
                    lo_r = 640 * q
                    hi_r = lo_r + SL[q]
                    base = NC * lo_r
                    nc.gpsimd.collective_compute(
                        "AllGather", mybir.AluOpType.bypass,
                        replica_groups=[list(range(NC))],
                        ins=[hg_self[lo_r:hi_r, :]],
                        outs=[hg_full[base:base + NC * SL[q], :]])

            if stage == "l1":
                for b in range(NBLK):
                    rows = min(128, NPC - b * 128)
                    nc.gpsimd.dma_start(
                        out=out_t[b * 128:b * 128 + rows, :],
                        in_=h_own[0:rows, b * DH:(b + 1) * DH])

            # ---- layer 2 ----
            for b in range(NBLK if stage == "full" else 0):
                nchunk = R[b] // 128
                o16 = int(offs[b]) // 16
                t0 = int(offs[b]) // 128
                oht = ohp.tile([128, smax, 128], bf16, tag="oh2")
                nc.sync.dma_start(oht[:, 0:nchunk, :],
                                  oh_in[:, t0:t0 + nchunk, :])
                gh = gxp.tile([128, smax, DH], bf16, tag="gh")
                nc.gpsimd.dma_gather(
                    out_ap=gh[:, 0:nchunk, :], in_ap=hg_full[:],
                    idxs_ap=idx2[:, o16:o16 + R[b] // 16],
                    num_idxs=R[b], num_idxs_reg=cnt_vals[b], elem_size=DH,
                    single_packet=False, queue_num=b % 4)
                psum_lo = ps_ag.tile([128, 128], f32, tag="agg", space="PSUM")
                psum_hi = ps_ag.tile([128, 128], f32, tag="agg", space="PSUM")
                for k in range(nchunk):
                    nc.tensor.matmul(out=psum_lo[:], lhsT=gh[:, k, 0:128],
                                     rhs=oht[:, k, :], start=(k == 0),
                                     stop=(k == nchunk - 1))
                    nc.tensor.matmul(out=psum_hi[:], lhsT=gh[:, k, 128:256],
                                     rhs=oht[:, k, :], start=(k == 0),
                                     stop=(k == nchunk - 1))
                z2lo = rot.tile([128, 128], bf16, tag="z2lo")
                nc.scalar.activation(z2lo[:], psum_lo[:],
                                     mybir.ActivationFunctionType.Copy)
                z2hi = rot.tile([128, 128], bf16, tag="z2hi")
                nc.scalar.activation(z2hi[:], psum_hi[:],
                                     mybir.ActivationFunctionType.Copy)
                psum_d2 = ps_dn.tile([128, DH], f32, tag="dense",
                                     space="PSUM")
                nc.tensor.matmul(out=psum_d2[:], lhsT=ones_t[:], rhs=b2row,
                                 start=True, stop=False)
                nc.tensor.matmul(out=psum_d2[:], lhsT=z2lo[:], rhs=W2ab,
                                 start=False, stop=False)
                nc.tensor.matmul(out=psum_d2[:], lhsT=z2hi[:], rhs=W2bb,
                                 start=False, stop=False)
                nc.tensor.matmul(out=psum_d2[:], lhsT=ident[:],
                                 rhs=h_own[:, b * DH:(b + 1) * DH],
                                 start=False, stop=True)
                outb = epi.tile([128, DH], f32, tag="outb")
                _ln(nc, epi, psum_d2, g2t, be2t, outb, eps_t[:])
                rows = min(128, NPC - b * 128)
                nc.sync.dma_start(out=out_t[b * 128:b * 128 + rows, :],
                                  in_=outb[0:rows, :])
    nc.compile()
    return nc


def _host_prep(edge_index, edge_weight):
    src = np.concatenate([np.asarray(edge_index[0], np.int64),
                          np.arange(N, dtype=np.int64)])
    dst = np.concatenate([np.asarray(edge_index[1], np.int64),
                          np.arange(N, dtype=np.int64)])
    w = np.concatenate([np.asarray(edge_weight, np.float32),
                        np.ones(N, np.float32)])
    deg = np.zeros(N, np.float32)
    np.add.at(deg, dst, w)
    dinv = np.where(deg > 0, 1.0 / np.sqrt(deg), 0.0).astype(np.float32)
    norm = (dinv[src] * w * dinv[dst]).astype(np.float32)

    order = np.argsort(dst, kind='stable')
    src_s, dst_s, norm_s = src[order], dst[order], norm[order]

    core_id = dst_s // NPC
    brel = (dst_s % NPC) // 128
    cnt = np.zeros((NC, NBLK), np.int64)
    np.add.at(cnt, (core_id, brel), 1)
    R = tuple(int(v) for v in (np.ceil(cnt.max(axis=0) / 128) * 128)
              .astype(np.int64))
    offs = np.concatenate([[0], np.cumsum(R)]).astype(np.int64)
    E_pad = int(offs[-1])

    src_pad = np.full((NC, E_pad), -1, np.int16)
    dstrel_pad = np.zeros((NC, E_pad), np.int64)
    wn_pad = np.zeros((NC, E_pad), np.float32)
    real = np.zeros((NC, E_pad), bool)
    for c in range(NC):
        for b in range(NBLK):
            lo = np.searchsorted(dst_s, c * NPC + b * 128, 'left')
            hi = np.searchsorted(
                dst_s, min(c * NPC + (b + 1) * 128, (c + 1) * NPC), 'left')
            n = hi - lo
            o = int(offs[b])
            src_pad[c, o:o + n] = src_s[lo:hi]
            dstrel_pad[c, o:o + n] = dst_s[lo:hi] - (c * NPC + b * 128)
            wn_pad[c, o:o + n] = norm_s[lo:hi]
            real[c, o:o + n] = True
    return R, cnt.astype(np.int32), src_pad, dstrel_pad, wn_pad, real


def kernel(x, edge_index, edge_weight, W1, b1, W2, b2, Wres, bres,
           gamma1, beta1, gamma2, beta2):
    global LAST_EXEC_TIME_NS
    x = np.ascontiguousarray(np.asarray(x, np.float32))
    W1 = np.asarray(W1, np.float32)
    W2 = np.asarray(W2, np.float32)
    Wres = np.asarray(Wres, np.float32)

    R, cnt, src_pad, dstrel_pad, wn_pad, real = _host_prep(
        edge_index, edge_weight)
    E_pad = int(sum(R))
    CH = E_pad // 128

    cstf = np.zeros((128, 8, DH), np.float32)
    cstf[:, F_WRES, :] = Wres
    cstf[:, F_B1, :] = np.asarray(b1, np.float32)[None, :]
    cstf[:, F_BRES, :] = np.asarray(bres, np.float32)[None, :]
    cstf[:, F_G1, :] = np.asarray(gamma1, np.float32)[None, :]
    cstf[:, F_BE1, :] = np.asarray(beta1, np.float32)[None, :]
    cstf[:, F_B2, :] = np.asarray(b2, np.float32)[None, :]
    cstf[:, F_G2, :] = np.asarray(gamma2, np.float32)[None, :]
    cstf[:, F_BE2, :] = np.asarray(beta2, np.float32)[None, :]
    cstb = np.zeros((128, 6, DH), np.float32)
    cstb[:, B_W1, :] = W1
    cstb[:, B_W2A, :] = W2[:128, :]
    cstb[:, B_W2B, :] = W2[128:, :]
    cstb[:, B_B1, :] = np.asarray(b1, np.float32)[None, :]
    cstb[:, B_BRES, :] = np.asarray(bres, np.float32)[None, :]
    cstb[:, B_B2, :] = np.asarray(b2, np.float32)[None, :]
    cstb = cstb.astype(ml_dtypes.bfloat16)

    xbf = x.astype(ml_dtypes.bfloat16)

    # row map for the sliced-exchange layout of hg_full:
    # node n = 2500c + l; slice q of l (640,640,640,580); row =
    # 8*640*q + SL[q]*c + (l - 640*q)
    SL = [640, 640, 640, NPC - 3 * 640]
    n_all = np.arange(N, dtype=np.int64)
    c_all = n_all // NPC
    l_all = n_all % NPC
    q_all = np.minimum(l_all // 640, 3)
    base_all = NC * 640 * q_all
    slq = np.asarray(SL, np.int64)[q_all]
    rowmap = (base_all + slq * c_all + (l_all - 640 * q_all)).astype(np.int16)

    in_maps = []
    for c in range(NC):
        idx_w = np.tile(src_pad[c].reshape(E_pad // 16, 16).T, (8, 1)).copy()
        src2 = src_pad[c].astype(np.int64)
        idx2_flat = np.where(src_pad[c] >= 0, rowmap[src2], -1).astype(np.int16)
        idx2_w = np.tile(idx2_flat.reshape(E_pad // 16, 16).T, (8, 1)).copy()
        # one-hot scatter tiles: [128, CH, 128] bf16
        oh = np.zeros((E_pad, 128), np.float32)
        rr = real[c]
        oh[np.nonzero(rr)[0], dstrel_pad[c][rr]] = wn_pad[c][rr]
        oh = np.ascontiguousarray(
            oh.reshape(CH, 128, 128).transpose(1, 0, 2)
        ).astype(ml_dtypes.bfloat16)
        xT = np.zeros((128, NBLK * 128), np.float32)
        xT[:, :NPC] = x[c * NPC:(c + 1) * NPC].T
        in_maps.append({
            "xbf": xbf,
            "idx1": idx_w,
            "idx2": idx2_w,
            "ohb": oh,
            "xT": xT,
            "cstf": cstf,
            "cstb": cstb,
            "cnt": cnt[c].reshape(1, NBLK),
        })

    stage = os.environ.get("GCN_STAGE", "full")
    key = (R, stage)
    nc = _prog_cache.get(key)
    if nc is None:
        nc = _build_program(R, stage)
        _prog_cache[key] = nc

    trace = bool(os.environ.get("BASS_KERNEL_TRACE"))
    res = run_bass_kernel_spmd(nc, in_maps, list(range(NC)), trace=trace)
    if trace:
        LAST_EXEC_TIME_NS = res.exec_time_ns
    out = np.concatenate([res.results[c]["out"] for c in range(NC)], axis=0)
    return np.ascontiguousarray(out.astype(np.float32))
